# revision 1
# baseline (speedup 1.0000x reference)
"""Trainium2 Bass kernel for NeighborAggregator (gather -> segment_sum -> softmax).

Strategy (8 NeuronCores, SPMD):
  - Row-shard the N=16384 instances: each core owns R=2048 rows = 65536 edges.
  - Stream the core's row shard through SBUF in 16 blocks of 128 rows
    (8MiB each); for each block gather d[p,j] = row_p[idx[p,j]] on-chip with
    gpsimd indirect_copy. indirect_copy applies each index to all 16
    partitions of a Q7 group, so the result is masked with a static
    "diagonal" mask and strided-reduced to keep only the owning partition's
    element.
  - contrib = d * values.
  - Segment-sum over C=16384 columns via PE outer-product histogram:
    for each tile of 128 edges (one per partition), build one-hot masks
      A[p,h] = contrib_p * [hi_p == h]   (hi = idx >> 7)
      B[p,l] = [lo_p == l]               (lo = idx & 127)
    and accumulate M += A^T @ B into a [128,128] f32 PSUM tile, which is
    exactly the per-core partial segment_sum laid out as c = 128*h + l.
  - AllReduce the [128,128] partials across the 8 cores, then compute the
    softmax on-device; every core writes the identical full outputs.
"""

import sys

for _p in ("/opt/trn_rl_repo", "/root/.axon_site/_ro/trn_rl_repo"):
    if _p not in sys.path:
        sys.path.append(_p)

import numpy as np

N = 16384
C = 16384
K = 32
NCORES = 8
P = 128

R = N // NCORES          # rows per core
NBLK = R // P            # row blocks per core
TB = 8                   # tiles per batched mask-build op
HALF = C // 2            # ap_gather data-size limit: 8192 f32 per partition

_CACHE = {}


def _build_program(debug_dumps=False):
    from concourse import bacc, bass, bass_isa, mybir, tile

    f32 = mybir.dt.float32
    i32 = mybir.dt.int32
    i16 = mybir.dt.int16

    nc = bacc.Bacc("TRN2", target_bir_lowering=False, debug=False,
                   num_devices=NCORES)

    rows_d = nc.dram_tensor("rows", [R, C], f32, kind="ExternalInput")
    idx_d = nc.dram_tensor("idx", [R, K], i32, kind="ExternalInput")
    vals_d = nc.dram_tensor("vals", [R, K], f32, kind="ExternalInput")
    alpha_d = nc.dram_tensor("alpha", [P, P], f32, kind="ExternalOutput")
    red_d = nc.dram_tensor("red", [P, P], f32, kind="ExternalOutput")
    if debug_dumps:
        gath_dump = nc.dram_tensor("gath_dump", [P, K], f32, kind="ExternalOutput")
        part_dump = nc.dram_tensor("part_dump", [P, P], f32, kind="ExternalOutput")

    with tile.TileContext(nc) as tc:
        with tc.tile_pool(name="fix", bufs=1) as fix, \
             tc.tile_pool(name="rows", bufs=2) as rows_pool, \
             tc.tile_pool(name="work", bufs=3) as work, \
             tc.tile_pool(name="psum", bufs=1, space="PSUM") as psum_pool, \
             tc.tile_pool(name="dram", bufs=1, space="DRAM") as dram_pool:

            # --- one-time constants ---
            # iota over the free dim (one-hot comparison target), as f32
            iota_i = fix.tile([P, P], i32)
            nc.gpsimd.iota(iota_i[:], pattern=[[1, P]], base=0,
                           channel_multiplier=0)
            iota_f = fix.tile([P, P], f32)
            nc.vector.tensor_copy(out=iota_f[:], in_=iota_i[:])

            # diagonal mask for indirect_copy extraction:
            # dmask[p, s*16 + k] = (k == p % 16)
            fmod = fix.tile([P, K * 16], i32)
            nc.gpsimd.iota(fmod[:], pattern=[[0, K], [1, 16]], base=0,
                           channel_multiplier=0)
            pidx = fix.tile([P, 1], i32)
            nc.gpsimd.iota(pidx[:], pattern=[[0, 1]], base=0,
                           channel_multiplier=1)
            pmod = fix.tile([P, 1], i32)
            nc.vector.tensor_scalar(out=pmod[:], in0=pidx[:], scalar1=15,
                                    scalar2=None, op0=mybir.AluOpType.bitwise_and)
            dmask = fix.tile([P, K * 16], f32)
            nc.vector.tensor_tensor(out=dmask[:], in0=fmod[:],
                                    in1=pmod[:].to_broadcast([P, K * 16]),
                                    op=mybir.AluOpType.is_equal)

            psum = psum_pool.tile([P, P], f32)

            for rb in range(NBLK):
                r0 = rb * P
                rows_blk = rows_pool.tile([P, C], f32, tag="rows")
                nc.sync.dma_start(out=rows_blk[:], in_=rows_d[r0:r0 + P, :])

                idx_blk = work.tile([P, K], i32, tag="idx")
                nc.sync.dma_start(out=idx_blk[:], in_=idx_d[r0:r0 + P, :])
                vals_blk = work.tile([P, K], f32, tag="vals")
                nc.sync.dma_start(out=vals_blk[:], in_=vals_d[r0:r0 + P, :])

                # ap_gather is limited to 32KB (=8192 f32) of data per
                # partition, so gather from each half-row and select.
                idx_a16 = work.tile([P, K], i16, tag="idxa")
                nc.vector.tensor_scalar(out=idx_a16[:], in0=idx_blk[:],
                                        scalar1=HALF - 1, scalar2=None,
                                        op0=mybir.AluOpType.min)
                idx_b16 = work.tile([P, K], i16, tag="idxb")
                nc.vector.tensor_scalar(out=idx_b16[:], in0=idx_blk[:],
                                        scalar1=HALF, scalar2=0,
                                        op0=mybir.AluOpType.subtract,
                                        op1=mybir.AluOpType.max)
                hmask = work.tile([P, K], f32, tag="hmask")
                nc.vector.tensor_scalar(out=hmask[:], in0=idx_blk[:],
                                        scalar1=HALF, scalar2=None,
                                        op0=mybir.AluOpType.is_ge)

                # on-chip gather (idx i applied to all 16 partitions of each
                # Q7 group): gath[p, s*16+k] = rows_blk[p, idx[(g*16)+k, s]]
                gath_a = work.tile([P, K * 16], f32, tag="gath_a")
                nc.gpsimd.ap_gather(out_ap=gath_a[:], in_ap=rows_blk[:, :HALF],
                                    idxs_ap=idx_a16[:], channels=P,
                                    num_elems=HALF, d=1, num_idxs=K * 16)
                gath_b = work.tile([P, K * 16], f32, tag="gath_b")
                nc.gpsimd.ap_gather(out_ap=gath_b[:], in_ap=rows_blk[:, HALF:],
                                    idxs_ap=idx_b16[:], channels=P,
                                    num_elems=HALF, d=1, num_idxs=K * 16)

                gsel_a = work.tile([P, K * 16], f32, tag="gsel_a")
                nc.vector.tensor_tensor(out=gsel_a[:], in0=gath_a[:],
                                        in1=dmask[:], op=mybir.AluOpType.mult)
                gsel_b = work.tile([P, K * 16], f32, tag="gsel_b")
                nc.vector.tensor_tensor(out=gsel_b[:], in0=gath_b[:],
                                        in1=dmask[:], op=mybir.AluOpType.mult)
                dv_a = work.tile([P, K], f32, tag="dv_a")
                nc.vector.tensor_reduce(
                    out=dv_a[:],
                    in_=gsel_a[:].rearrange("p (s k) -> p s k", k=16),
                    axis=mybir.AxisListType.X,
                    op=mybir.AluOpType.add)
                dv_b = work.tile([P, K], f32, tag="dv_b")
                nc.vector.tensor_reduce(
                    out=dv_b[:],
                    in_=gsel_b[:].rearrange("p (s k) -> p s k", k=16),
                    axis=mybir.AxisListType.X,
                    op=mybir.AluOpType.add)

                # d = dv_a + (dv_b - dv_a) * hmask
                ddiff = work.tile([P, K], f32, tag="ddiff")
                nc.vector.tensor_tensor(out=ddiff[:], in0=dv_b[:], in1=dv_a[:],
                                        op=mybir.AluOpType.subtract)
                dsel = work.tile([P, K], f32, tag="dsel")
                nc.vector.tensor_tensor(out=dsel[:], in0=ddiff[:], in1=hmask[:],
                                        op=mybir.AluOpType.mult)
                dvals = work.tile([P, K], f32, tag="dvals")
                nc.vector.tensor_tensor(out=dvals[:], in0=dsel[:], in1=dv_a[:],
                                        op=mybir.AluOpType.add)
                contrib = work.tile([P, K], f32, tag="contrib")
                nc.vector.tensor_tensor(out=contrib[:], in0=dvals[:],
                                        in1=vals_blk[:],
                                        op=mybir.AluOpType.mult)
                if debug_dumps and rb == 0:
                    nc.sync.dma_start(out=gath_dump[:], in_=dvals[:])

                hi_i = work.tile([P, K], i32, tag="hi_i")
                nc.vector.tensor_scalar(out=hi_i[:], in0=idx_blk[:], scalar1=7,
                                        scalar2=None,
                                        op0=mybir.AluOpType.logical_shift_right)
                lo_i = work.tile([P, K], i32, tag="lo_i")
                nc.vector.tensor_scalar(out=lo_i[:], in0=idx_blk[:],
                                        scalar1=127, scalar2=None,
                                        op0=mybir.AluOpType.bitwise_and)
                hif = work.tile([P, K], f32, tag="hif")
                nc.vector.tensor_copy(out=hif[:], in_=hi_i[:])
                lof = work.tile([P, K], f32, tag="lof")
                nc.vector.tensor_copy(out=lof[:], in_=lo_i[:])

                for t0 in range(0, K, TB):
                    hi_b = hif[:, t0:t0 + TB][:, :, None].broadcast_to([P, TB, P])
                    lo_b = lof[:, t0:t0 + TB][:, :, None].broadcast_to([P, TB, P])
                    ct_b = contrib[:, t0:t0 + TB][:, :, None].broadcast_to([P, TB, P])
                    io_b = iota_f[:][:, None, :].broadcast_to([P, TB, P])

                    a_eq = work.tile([P, TB, P], f32, tag="a_eq")
                    nc.vector.tensor_tensor(out=a_eq[:], in0=hi_b, in1=io_b,
                                            op=mybir.AluOpType.is_equal)
                    a_m = work.tile([P, TB, P], f32, tag="a_m")
                    nc.vector.tensor_tensor(out=a_m[:], in0=a_eq[:], in1=ct_b,
                                            op=mybir.AluOpType.mult)
                    b_eq = work.tile([P, TB, P], f32, tag="b_eq")
                    nc.vector.tensor_tensor(out=b_eq[:], in0=lo_b, in1=io_b,
                                            op=mybir.AluOpType.is_equal)

                    for t in range(TB):
                        ti = rb * K + t0 + t
                        nc.tensor.matmul(
                            out=psum[:],
                            lhsT=a_m[:, t, :],
                            rhs=b_eq[:, t, :],
                            start=(ti == 0),
                            stop=(ti == NBLK * K - 1),
                        )

            red_sb = fix.tile([P, P], f32)
            nc.vector.tensor_copy(out=red_sb[:], in_=psum[:])

            if debug_dumps:
                nc.sync.dma_start(out=part_dump[:], in_=red_sb[:])

            partial = dram_pool.tile([P, P], f32)
            nc.sync.dma_start(out=partial[:], in_=red_sb[:])
            allred = dram_pool.tile([P, P], f32)
            nc.gpsimd.collective_compute(
                "AllReduce",
                mybir.AluOpType.add,
                replica_groups=[list(range(NCORES))],
                ins=[partial[:].opt()],
                outs=[allred[:].opt()],
            )
            r_sb = fix.tile([P, P], f32)
            nc.sync.dma_start(out=r_sb[:], in_=allred[:])
            nc.sync.dma_start(out=red_d[:], in_=r_sb[:])

            # softmax over all 16384 entries of r_sb
            pm = fix.tile([P, 1], f32)
            nc.vector.tensor_reduce(out=pm[:], in_=r_sb[:],
                                    axis=mybir.AxisListType.X,
                                    op=mybir.AluOpType.max)
            gm = fix.tile([P, 1], f32)
            nc.gpsimd.partition_all_reduce(gm[:], pm[:], channels=P,
                                           reduce_op=bass_isa.ReduceOp.max)
            negm = fix.tile([P, 1], f32)
            nc.vector.tensor_scalar_mul(negm[:], gm[:], -1.0)
            e_sb = fix.tile([P, P], f32)
            s_sb = fix.tile([P, 1], f32)
            nc.scalar.activation(out=e_sb[:], in_=r_sb[:],
                                 func=mybir.ActivationFunctionType.Exp,
                                 bias=negm[:], scale=1.0, accum_out=s_sb[:])
            stot = fix.tile([P, 1], f32)
            nc.gpsimd.partition_all_reduce(stot[:], s_sb[:], channels=P,
                                           reduce_op=bass_isa.ReduceOp.add)
            rec = fix.tile([P, 1], f32)
            nc.vector.reciprocal(rec[:], stot[:])
            alpha_sb = fix.tile([P, P], f32)
            nc.scalar.activation(out=alpha_sb[:], in_=e_sb[:],
                                 func=mybir.ActivationFunctionType.Copy,
                                 scale=rec[:])
            nc.sync.dma_start(out=alpha_d[:], in_=alpha_sb[:])

    nc.compile()
    return nc


def _get_program():
    if "nc" not in _CACHE:
        _CACHE["nc"] = _build_program()
    return _CACHE["nc"]


def make_in_maps(input_tensor, indices, values):
    input_tensor = np.ascontiguousarray(np.asarray(input_tensor, dtype=np.float32))
    indices = np.asarray(indices)
    values = np.ascontiguousarray(np.asarray(values, dtype=np.float32))
    in_maps = []
    for m in range(NCORES):
        r0, r1 = m * R, (m + 1) * R
        in_maps.append({
            "rows": input_tensor[r0:r1],
            "idx": np.ascontiguousarray(indices[r0:r1].astype(np.int32)),
            "vals": values[r0:r1],
        })
    return in_maps


def kernel(input_tensor, indices, values, k=K, **_unused):
    assert int(k) == K
    from concourse.bass_utils import run_bass_kernel_spmd

    nc = _get_program()
    in_maps = make_in_maps(input_tensor, indices, values)
    res = run_bass_kernel_spmd(nc, in_maps, list(range(NCORES)))
    out0 = res.results[0]
    alpha = np.asarray(out0["alpha"], dtype=np.float32).reshape(C)
    reduced = np.asarray(out0["red"], dtype=np.float32).reshape(C)
    return alpha, reduced

